# revision 1
# baseline (speedup 1.0000x reference)
"""Multi-head attention (B=2, S=2048, D=1024, H=16) on 8 trn2 NeuronCores.

Sharding: data-parallel over batch (2) x tensor-parallel over head-groups (4).
Core c handles batch c//4, heads [4*(c%4), 4*(c%4)+4).  Each core computes
q/k/v projections for its 256 head-features, masked softmax attention for its
4 heads, and the row-shard of the output projection; partial outputs are
summed on the host during the gather step.
"""

import sys
import functools
from contextlib import ExitStack

sys.path.insert(0, "/opt/trn_rl_repo")

import numpy as np

B, S, D, H = 2, 2048, 1024, 16
DH = 64
P = 128
NCORES = 8
GROUPS = 4            # head groups = cores per batch
NH = H // GROUPS      # heads per core = 4
F = NH * DH           # head features per core = 256
KS = S // P           # 16 key slices
QT = S // P           # 16 query tiles
DS = D // P           # 8 d_model slices
VW = DH + 1           # v width per head incl. ones column = 65


def _emit(nc, tc, t, dbg=None):
    import concourse.mybir as mybir
    bf16 = mybir.dt.bfloat16
    f32 = mybir.dt.float32
    Exp = mybir.ActivationFunctionType.Exp
    Copy = mybir.ActivationFunctionType.Copy

    es = ExitStack()
    const_pool = es.enter_context(tc.tile_pool(name="const", bufs=1))
    w_pool = es.enter_context(tc.tile_pool(name="w", bufs=1))
    x_pool = es.enter_context(tc.tile_pool(name="x", bufs=2))
    qk_pool = es.enter_context(tc.tile_pool(name="qk", bufs=1))
    v_pool = es.enter_context(tc.tile_pool(name="v", bufs=1))
    nm_pool = es.enter_context(tc.tile_pool(name="nm", bufs=3))
    pt_pool = es.enter_context(tc.tile_pool(name="pt", bufs=7))
    ctxT_pool = es.enter_context(tc.tile_pool(name="ctxT", bufs=1))
    rs_pool = es.enter_context(tc.tile_pool(name="rs", bufs=4))
    ps_pool = es.enter_context(tc.tile_pool(name="ps", bufs=2, space="PSUM"))
    psp_cm = tc.tile_pool(name="psproj", bufs=4, space="PSUM")
    psp_pool = psp_cm.__enter__()

    bq_sb = const_pool.tile([P, 2], f32, tag="bq")
    bk_sb = const_pool.tile([P, 2], f32, tag="bk")

    # Weights: [128, DS*F] layout, col = ds*F + f
    wq_sb = w_pool.tile([P, DS * F], bf16, tag="wq")
    wk_sb = w_pool.tile([P, DS * F], bf16, tag="wk")
    wv_sb = w_pool.tile([P, DS * F], bf16, tag="wv")
    wom_sb = w_pool.tile([P, 2 * D], bf16, tag="wom")
    def load_w(w_sb, wname, ng):
        nc.sync.dma_start(
            w_sb[:].rearrange("p (g f) -> p g f", g=ng),
            t[wname].ap().rearrange("(g p) f -> p g f", p=P),
        )

    load_w(wq_sb, "wqT", DS)
    nc.sync.dma_start(bq_sb[:], t["bq"].ap().rearrange("(ft p) one -> p (ft one)", p=P))
    nc.sync.dma_start(bk_sb[:], t["bk"].ap().rearrange("(ft p) one -> p (ft one)", p=P))

    # ---- Phase A: projections ----
    # qT/kT [256, S] bf16 as 2 tiles of [128, S] (partition = head-feature)
    qT = [qk_pool.tile([P, S], bf16, tag=f"qT{ft}", name=f"qT{ft}") for ft in range(2)]
    kT = [qk_pool.tile([P, S], bf16, tag=f"kT{ft}", name=f"kT{ft}") for ft in range(2)]
    # v [token, feat] with per-head ones column: [128, KS * NH * VW]
    v_sb = v_pool.tile([P, KS * NH * VW], bf16, tag="v")
    nc.gpsimd.memset(v_sb[:], 1.0)

    def load_x(x_name):
        x_sb = x_pool.tile([P, DS * S], bf16, tag="xfull", name=f"x_{x_name}")
        # tc2-major chunking: the first accumulation groups only need the
        # first 1024-column chunk of every d-slice, so they can start after
        # ~2MB of DMA instead of the full 4MB tensor.
        for tc2 in range(2):
            for dh in range(2):
                nc.sync.dma_start(
                    x_sb[:].rearrange("p (ds c) -> p ds c", ds=DS)[:, dh * 4:(dh + 1) * 4, tc2 * 1024:(tc2 + 1) * 1024],
                    t[x_name].ap().rearrange("(ds p) c -> p ds c", p=P)[:, dh * 4:(dh + 1) * 4, tc2 * 1024:(tc2 + 1) * 1024],
                )
        return x_sb

    def qk_proj_ft(x_sb, w_sb, b_sb, out_tiles, ft):
        for tc4 in range(4):
            ps = psp_pool.tile([P, 512], f32, tag="psproj")
            for ds in range(DS):
                nc.tensor.matmul(
                    ps[:],
                    w_sb[:, ds * F + ft * P: ds * F + (ft + 1) * P],
                    x_sb[:, ds * S + tc4 * 512: ds * S + (tc4 + 1) * 512],
                    start=(ds == 0),
                    stop=(ds == DS - 1),
                )
            nc.vector.tensor_scalar_add(
                out_tiles[ft][:, tc4 * 512:(tc4 + 1) * 512], ps[:], b_sb[:, ft:ft + 1]
            )

    xq_sb = load_x("xqT")
    qk_proj_ft(xq_sb, wq_sb, bq_sb, qT, 0)
    load_w(wv_sb, "wvT", DS)
    qk_proj_ft(xq_sb, wq_sb, bq_sb, qT, 1)
    load_w(wk_sb, "wkT", DS)
    load_w(wom_sb, "womT", 2)

    # v projection before k: the attention stream (scores -> exp) starts the
    # moment kT is done, with no v-projection bubble on the ACT engine.
    xv_sb = x_pool.tile([P, DS * S], bf16, tag="xfull", name="x_xvT")
    for tc2 in range(2):
        for dh in range(2):
            nc.sync.dma_start(
                xv_sb[:].rearrange("p (ds c) -> p ds c", ds=DS)[:, dh * 4:(dh + 1) * 4, tc2 * 1024:(tc2 + 1) * 1024],
                t["xvT"].ap().rearrange("(ds p) c -> p ds c", p=P)[:, dh * 4:(dh + 1) * 4, tc2 * 1024:(tc2 + 1) * 1024],
            )
    for tt in range(KS):
        ps = psp_pool.tile([P, F], f32, tag="psproj")
        for ds in range(DS):
            nc.tensor.matmul(
                ps[:],
                xv_sb[:, ds * S + tt * P: ds * S + tt * P + P],
                wv_sb[:, ds * F:(ds + 1) * F],
                start=(ds == 0),
                stop=(ds == DS - 1),
            )
        dst = v_sb[:, tt * NH * VW:(tt + 1) * NH * VW].rearrange("p (h w) -> p h w", h=NH)[:, :, 0:DH]
        nc.vector.tensor_copy(dst, ps[:].rearrange("p (h f) -> p h f", h=NH))
    if dbg is not None:
        nc.sync.dma_start(dbg["v"].ap(), v_sb[:])

    xk_sb = load_x("xkT")
    qk_proj_ft(xk_sb, wk_sb, bk_sb, kT, 0)
    qk_proj_ft(xk_sb, wk_sb, bk_sb, kT, 1)
    if dbg is not None:
        for ft in range(2):
            nc.sync.dma_start(dbg["qT"].ap()[ft * P:(ft + 1) * P, :], qT[ft][:])
            nc.sync.dma_start(dbg["kT"].ap()[ft * P:(ft + 1) * P, :], kT[ft][:])

    psp_cm.__exit__(None, None, None)

    # ---- Phase B setup (scores closures; ctx^T accumulation) ----
    # ctx accumulated transposed: ctx^T[f, q] = sum_k v_ext[k, f] * pT[k, q]
    # (stationary = v slice, moving = pT strips -> few wide matmuls instead of
    # many narrow ones; the PE sequencer dispatch rate is the limit otherwise).
    # Row DH of ctx^T is the softmax denominator via the ones column of v_ext.
    ctxT = [ctxT_pool.tile([P, S], bf16, tag=f"ctxT{h}", name=f"ctxT{h}") for h in range(2)]
    ctx_cm = tc.tile_pool(name="ctxps", bufs=1, space="PSUM")
    ctx_pool = ctx_cm.__enter__()
    all_pts = {h: [None] * KS for h in range(NH)}

    def scores(h, ks):
        ft, ro = h // 2, (h % 2) * DH
        pt = pt_pool.tile([P, S], bf16, tag="pt")
        nm = nm_pool.tile([P, S], bf16, tag="nm")
        nc.gpsimd.dma_start(nm[:], t["nmT"].ap()[ks * P:(ks + 1) * P, :])
        for qc in range(2):
            ps = ps_pool.tile([P, 1024], f32, tag="ps")
            for qh in range(2):
                nc.tensor.matmul(
                    ps[:, qh * 512:(qh + 1) * 512],
                    kT[ft][ro:ro + DH, ks * P:(ks + 1) * P],
                    qT[ft][ro:ro + DH, qc * 1024 + qh * 512: qc * 1024 + (qh + 1) * 512],
                    start=True,
                    stop=True,
                )
            nc.scalar.activation(pt[:, qc * 1024:(qc + 1) * 1024], ps[:], Exp, scale=0.125)
            nc.vector.tensor_mul(
                pt[:, qc * 1024:(qc + 1) * 1024],
                pt[:, qc * 1024:(qc + 1) * 1024],
                nm[:, qc * 1024:(qc + 1) * 1024],
            )
        all_pts[h][ks] = pt
        if dbg is not None and h == 0:
            nc.sync.dma_start(dbg["pT0"].ap()[ks * P:(ks + 1) * P, :], pt[:])

    EARLY = 0

    # ---- Phase B: attention per head, software-pipelined over key slices ----
    DEPTH = 5  # software-pipeline offset between scores and ctx
    for h in range(NH):
        ft, ro = h // 2, (h % 2) * DH
        ctx_ps = ctx_pool.tile([P, S], f32, tag="ctx")
        pts = all_pts[h]

        def ctx_step(ks):
            pt = pts[ks]
            for qc4 in range(4):
                nc.tensor.matmul(
                    ctx_ps[0:VW, qc4 * 512:(qc4 + 1) * 512],
                    v_sb[:, ks * NH * VW + h * VW: ks * NH * VW + (h + 1) * VW],
                    pt[:, qc4 * 512:(qc4 + 1) * 512],
                    start=(ks == 0),
                    stop=(ks == KS - 1),
                )

        for ks in range(KS + DEPTH):
            if ks < KS and not (h == 0 and ks < EARLY):
                scores(h, ks)
            if ks >= DEPTH:
                ctx_step(ks - DEPTH)

        # normalize: ctxT[f, q] = ctx^T[f, q] / ctx^T[DH, q], in two column
        # chunks so the output projection can start on the first half.
        for nh2 in range(2):
            cs = slice(nh2 * (S // 2), (nh2 + 1) * (S // 2))
            rs = rs_pool.tile([1, S // 2], f32, tag="rs")
            nc.vector.reciprocal(rs[:], ctx_ps[DH:DH + 1, cs])
            rsb = rs_pool.tile([DH, S // 2], f32, tag="rsb")
            nc.gpsimd.partition_broadcast(rsb[:], rs[:])
            nc.vector.tensor_mul(ctxT[ft][ro:ro + DH, cs], ctx_ps[0:DH, cs], rsb[:])

    if dbg is not None:
        for half in range(2):
            nc.sync.dma_start(dbg["ctxT"].ap()[half * P:(half + 1) * P, :], ctxT[half][:])
    ctx_cm.__exit__(None, None, None)

    # ---- Phase D: output projection outT[o, q] = sum_f womT[f, o] * ctxT[f, q] ----
    # opj tiles come from ps_pool: its slots free up right after the last
    # scores' exp, so the first outproj matmuls don't wait on the ctx-pool
    # banks (which are only released once head 3's norm completes).
    out_cm = tc.tile_pool(name="out", bufs=4)
    out_pool = out_cm.__enter__()
    for qc in range(2):
        for ot in range(8):
            ps = ps_pool.tile([P, 1024], f32, tag="ps")
            for fs in range(2):
                for qh in range(2):
                    nc.tensor.matmul(
                        ps[:, qh * 512:(qh + 1) * 512],
                        wom_sb[:, fs * D + ot * P: fs * D + (ot + 1) * P],
                        ctxT[fs][:, qc * 1024 + qh * 512: qc * 1024 + (qh + 1) * 512],
                        start=(fs == 0),
                        stop=(fs == 1),
                    )
            osb = out_pool.tile([P, 1024], f32, tag="osb")
            if (ot * 2 + qc) % 2 == 0:
                nc.vector.tensor_copy(osb[:], ps[:])
            else:
                nc.scalar.copy(osb[:], ps[:])
            for qh in range(2):
                nc.sync.dma_start(
                    t["outT"].ap()[ot * P:(ot + 1) * P, qc * 1024 + qh * 512: qc * 1024 + (qh + 1) * 512],
                    osb[:, qh * 512:(qh + 1) * 512],
                )
    out_cm.__exit__(None, None, None)
    es.close()


@functools.lru_cache(maxsize=1)
def _build(debug=False):
    import concourse.bacc as bacc
    import concourse.mybir as mybir
    import concourse.tile as tile

    bf16 = mybir.dt.bfloat16
    f32 = mybir.dt.float32

    nc = bacc.Bacc("TRN2", target_bir_lowering=False, debug=False, num_devices=NCORES)
    t = {
        "xqT": nc.dram_tensor("xqT", (D, S), bf16, kind="ExternalInput"),
        "xkT": nc.dram_tensor("xkT", (D, S), bf16, kind="ExternalInput"),
        "xvT": nc.dram_tensor("xvT", (D, S), bf16, kind="ExternalInput"),
        "wqT": nc.dram_tensor("wqT", (D, F), bf16, kind="ExternalInput"),
        "wkT": nc.dram_tensor("wkT", (D, F), bf16, kind="ExternalInput"),
        "wvT": nc.dram_tensor("wvT", (D, F), bf16, kind="ExternalInput"),
        "womT": nc.dram_tensor("womT", (F, D), bf16, kind="ExternalInput"),
        "nmT": nc.dram_tensor("nmT", (S, S), bf16, kind="ExternalInput"),
        "bq": nc.dram_tensor("bq", (F, 1), f32, kind="ExternalInput"),
        "bk": nc.dram_tensor("bk", (F, 1), f32, kind="ExternalInput"),
        "outT": nc.dram_tensor("outT", (D, S), f32, kind="ExternalOutput"),
    }
    dbg = None
    if debug:
        dbg = {
            "qT": nc.dram_tensor("dbg_qT", (F, S), bf16, kind="ExternalOutput"),
            "kT": nc.dram_tensor("dbg_kT", (F, S), bf16, kind="ExternalOutput"),
            "v": nc.dram_tensor("dbg_v", (P, KS * NH * VW), bf16, kind="ExternalOutput"),
            "pT0": nc.dram_tensor("dbg_pT0", (S, S), bf16, kind="ExternalOutput"),
            "ctxT": nc.dram_tensor("dbg_ctxT", (F, S), bf16, kind="ExternalOutput"),
        }
    with tile.TileContext(nc) as tc:
        _emit(nc, tc, t, dbg)
    nc.compile()
    return nc


def _prep_core_inputs(c, Q, K, V, mask, Wq, bq, Wk, bk, Wv, Wo, _cache={}):
    import ml_dtypes

    bf = ml_dtypes.bfloat16
    b, g = divmod(c, GROUPS)
    bkey = ("batch", b, id(Q))
    if bkey not in _cache:
        _cache.clear()
        for bb in range(B):
            nm = 1.0 - mask[bb, 0].astype(np.float32)
            _cache[("batch", bb, id(Q))] = {
                "xqT": Q[bb].T.astype(bf),
                "xkT": K[bb].T.astype(bf),
                "xvT": V[bb].T.astype(bf),
                "nmT": nm.T.astype(bf),
            }
    fsl = slice(g * F, (g + 1) * F)
    return {
        **_cache[bkey],
        "wqT": Wq[fsl, :].T.astype(bf),
        "wkT": Wk[fsl, :].T.astype(bf),
        "wvT": Wv[fsl, :].T.astype(bf),
        "womT": Wo[:, fsl].T.astype(bf),
        "bq": bq[fsl].reshape(F, 1).astype(np.float32),
        "bk": bk[fsl].reshape(F, 1).astype(np.float32),
    }


def kernel(Q, K, V, mask, Wq, bq, Wk, bk, Wv, bv, Wo, bo, _trace=False, _tmpdir=None):
    from concourse.bass_utils import run_bass_kernel_spmd

    Q, K, V = np.asarray(Q, np.float32), np.asarray(K, np.float32), np.asarray(V, np.float32)
    mask = np.asarray(mask)
    Wq, Wk, Wv, Wo = (np.asarray(w, np.float32) for w in (Wq, Wk, Wv, Wo))
    bq, bk, bv, bo = (np.asarray(x, np.float32) for x in (bq, bk, bv, bo))

    nc = _build()
    in_maps = [_prep_core_inputs(c, Q, K, V, mask, Wq, bq, Wk, bk, Wv, Wo) for c in range(NCORES)]
    kw = {}
    if _trace:
        kw = dict(trace=True, tmpdir=_tmpdir)
    res = run_bass_kernel_spmd(nc, in_maps, core_ids=list(range(NCORES)), **kw)

    const = (Wo @ bv + bo).astype(np.float32)  # softmax rows sum to 1 -> bv enters linearly
    out = np.empty((B, S, D), np.float32)
    for b in range(B):
        acc = res.results[b * GROUPS]["outT"].astype(np.float32)
        for g in range(1, GROUPS):
            acc = acc + res.results[b * GROUPS + g]["outT"]
        out[b] = acc.T + const
    if _trace:
        kernel._last_results = res
    return out



# revision 19
# speedup vs baseline: 1.0086x; 1.0086x over previous
"""Multi-head attention (B=2, S=2048, D=1024, H=16) on 8 trn2 NeuronCores.

Sharding: data-parallel over batch (2) x tensor-parallel over head-groups (4).
Core c handles batch c//4, heads [4*(c%4), 4*(c%4)+4).  Each core computes
q/k/v projections for its 256 head-features, masked softmax attention for its
4 heads, and the row-shard of the output projection; partial outputs are
summed on the host during the gather step.
"""

import sys
import functools
from contextlib import ExitStack

sys.path.insert(0, "/opt/trn_rl_repo")

import numpy as np

B, S, D, H = 2, 2048, 1024, 16
DH = 64
P = 128
NCORES = 8
GROUPS = 4            # head groups = cores per batch
NH = H // GROUPS      # heads per core = 4
F = NH * DH           # head features per core = 256
KS = S // P           # 16 key slices
QT = S // P           # 16 query tiles
DS = D // P           # 8 d_model slices
VW = DH + 1           # v width per head incl. ones column = 65


def _emit(nc, tc, t, dbg=None):
    import concourse.mybir as mybir
    bf16 = mybir.dt.bfloat16
    f32 = mybir.dt.float32
    Exp = mybir.ActivationFunctionType.Exp
    Copy = mybir.ActivationFunctionType.Copy

    es = ExitStack()
    const_pool = es.enter_context(tc.tile_pool(name="const", bufs=1))
    w_pool = es.enter_context(tc.tile_pool(name="w", bufs=1))
    x_pool = es.enter_context(tc.tile_pool(name="x", bufs=2))
    qk_pool = es.enter_context(tc.tile_pool(name="qk", bufs=1))
    v_pool = es.enter_context(tc.tile_pool(name="v", bufs=1))
    nm_pool = es.enter_context(tc.tile_pool(name="nm", bufs=3))
    pt_pool = es.enter_context(tc.tile_pool(name="pt", bufs=7))
    ctxT_pool = es.enter_context(tc.tile_pool(name="ctxT", bufs=1))
    rs_pool = es.enter_context(tc.tile_pool(name="rs", bufs=4))
    ps_pool = es.enter_context(tc.tile_pool(name="ps", bufs=2, space="PSUM"))
    psp_cm = tc.tile_pool(name="psproj", bufs=4, space="PSUM")
    psp_pool = psp_cm.__enter__()

    bq_sb = const_pool.tile([P, 2], f32, tag="bq")
    bk_sb = const_pool.tile([P, 2], f32, tag="bk")

    # Weights: [128, DS*F] layout, col = ds*F + f
    wq_sb = w_pool.tile([P, DS * F], bf16, tag="wq")
    wk_sb = w_pool.tile([P, DS * F], bf16, tag="wk")
    wv_sb = w_pool.tile([P, DS * F], bf16, tag="wv")
    wom_sb = w_pool.tile([P, 2 * D], bf16, tag="wom")
    def load_w(w_sb, wname, ng):
        nc.sync.dma_start(
            w_sb[:].rearrange("p (g f) -> p g f", g=ng),
            t[wname].ap().rearrange("(g p) f -> p g f", p=P),
        )

    load_w(wq_sb, "wqT", DS)
    nc.sync.dma_start(bq_sb[:], t["bq"].ap().rearrange("(ft p) one -> p (ft one)", p=P))
    nc.sync.dma_start(bk_sb[:], t["bk"].ap().rearrange("(ft p) one -> p (ft one)", p=P))

    # ---- Phase A: projections ----
    # qT/kT [256, S] bf16 as 2 tiles of [128, S] (partition = head-feature)
    qT = [qk_pool.tile([P, S], bf16, tag=f"qT{ft}", name=f"qT{ft}") for ft in range(2)]
    kT = [qk_pool.tile([P, S], bf16, tag=f"kT{ft}", name=f"kT{ft}") for ft in range(2)]
    # v [token, feat] with per-head ones column: [128, KS * NH * VW]
    v_sb = v_pool.tile([P, KS * NH * VW], bf16, tag="v")
    nc.gpsimd.memset(v_sb[:], 1.0)

    def load_x(x_name):
        x_sb = x_pool.tile([P, DS * S], bf16, tag="xfull", name=f"x_{x_name}")
        # tc2-major chunking: the first accumulation groups only need the
        # first 1024-column chunk of every d-slice, so they can start after
        # ~2MB of DMA instead of the full 4MB tensor.
        for tc2 in range(2):
            for dh in range(2):
                nc.sync.dma_start(
                    x_sb[:].rearrange("p (ds c) -> p ds c", ds=DS)[:, dh * 4:(dh + 1) * 4, tc2 * 1024:(tc2 + 1) * 1024],
                    t[x_name].ap().rearrange("(ds p) c -> p ds c", p=P)[:, dh * 4:(dh + 1) * 4, tc2 * 1024:(tc2 + 1) * 1024],
                )
        return x_sb

    def qk_proj_ft(x_sb, w_sb, b_sb, out_tiles, ft):
        for tc4 in range(4):
            ps = psp_pool.tile([P, 512], f32, tag="psproj")
            for ds in range(DS):
                nc.tensor.matmul(
                    ps[:],
                    w_sb[:, ds * F + ft * P: ds * F + (ft + 1) * P],
                    x_sb[:, ds * S + tc4 * 512: ds * S + (tc4 + 1) * 512],
                    start=(ds == 0),
                    stop=(ds == DS - 1),
                )
            nc.vector.tensor_scalar_add(
                out_tiles[ft][:, tc4 * 512:(tc4 + 1) * 512], ps[:], b_sb[:, ft:ft + 1]
            )

    xq_sb = load_x("xqT")
    qk_proj_ft(xq_sb, wq_sb, bq_sb, qT, 0)
    load_w(wv_sb, "wvT", DS)
    qk_proj_ft(xq_sb, wq_sb, bq_sb, qT, 1)
    load_w(wk_sb, "wkT", DS)
    load_w(wom_sb, "womT", 2)

    # v projection before k: the attention stream (scores -> exp) starts the
    # moment kT is done, with no v-projection bubble on the ACT engine.
    xv_sb = x_pool.tile([P, DS * S], bf16, tag="xfull", name="x_xvT")
    for tc2 in range(2):
        for dh in range(2):
            nc.sync.dma_start(
                xv_sb[:].rearrange("p (ds c) -> p ds c", ds=DS)[:, dh * 4:(dh + 1) * 4, tc2 * 1024:(tc2 + 1) * 1024],
                t["xvT"].ap().rearrange("(ds p) c -> p ds c", p=P)[:, dh * 4:(dh + 1) * 4, tc2 * 1024:(tc2 + 1) * 1024],
            )
    for tt in range(KS):
        ps = psp_pool.tile([P, F], f32, tag="psproj")
        for ds in range(DS):
            nc.tensor.matmul(
                ps[:],
                xv_sb[:, ds * S + tt * P: ds * S + tt * P + P],
                wv_sb[:, ds * F:(ds + 1) * F],
                start=(ds == 0),
                stop=(ds == DS - 1),
            )
        dst = v_sb[:, tt * NH * VW:(tt + 1) * NH * VW].rearrange("p (h w) -> p h w", h=NH)[:, :, 0:DH]
        nc.vector.tensor_copy(dst, ps[:].rearrange("p (h f) -> p h f", h=NH))
    if dbg is not None:
        nc.sync.dma_start(dbg["v"].ap(), v_sb[:])

    xk_sb = load_x("xkT")
    qk_proj_ft(xk_sb, wk_sb, bk_sb, kT, 0)
    qk_proj_ft(xk_sb, wk_sb, bk_sb, kT, 1)
    if dbg is not None:
        for ft in range(2):
            nc.sync.dma_start(dbg["qT"].ap()[ft * P:(ft + 1) * P, :], qT[ft][:])
            nc.sync.dma_start(dbg["kT"].ap()[ft * P:(ft + 1) * P, :], kT[ft][:])

    psp_cm.__exit__(None, None, None)

    # ---- Phase B setup (scores closures; ctx^T accumulation) ----
    # ctx accumulated transposed: ctx^T[f, q] = sum_k v_ext[k, f] * pT[k, q]
    # (stationary = v slice, moving = pT strips -> few wide matmuls instead of
    # many narrow ones; the PE sequencer dispatch rate is the limit otherwise).
    # Row DH of ctx^T is the softmax denominator via the ones column of v_ext.
    ctxT = [ctxT_pool.tile([P, S], bf16, tag=f"ctxT{h}", name=f"ctxT{h}") for h in range(2)]
    ctx_cm = tc.tile_pool(name="ctxps", bufs=1, space="PSUM")
    ctx_pool = ctx_cm.__enter__()
    all_pts = {h: [None] * KS for h in range(NH)}

    def scores(h, ks):
        ft, ro = h // 2, (h % 2) * DH
        pt = pt_pool.tile([P, S], bf16, tag="pt")
        nm = nm_pool.tile([P, S], bf16, tag="nm")
        nc.gpsimd.dma_start(nm[:], t["nmT"].ap()[ks * P:(ks + 1) * P, :])
        for qc in range(2):
            ps = ps_pool.tile([P, 1024], f32, tag="ps")
            for qh in range(2):
                nc.tensor.matmul(
                    ps[:, qh * 512:(qh + 1) * 512],
                    kT[ft][ro:ro + DH, ks * P:(ks + 1) * P],
                    qT[ft][ro:ro + DH, qc * 1024 + qh * 512: qc * 1024 + (qh + 1) * 512],
                    start=True,
                    stop=True,
                )
            nc.scalar.activation(pt[:, qc * 1024:(qc + 1) * 1024], ps[:], Exp, scale=0.125)
            nc.vector.tensor_mul(
                pt[:, qc * 1024:(qc + 1) * 1024],
                pt[:, qc * 1024:(qc + 1) * 1024],
                nm[:, qc * 1024:(qc + 1) * 1024],
            )
        all_pts[h][ks] = pt
        if dbg is not None and h == 0:
            nc.sync.dma_start(dbg["pT0"].ap()[ks * P:(ks + 1) * P, :], pt[:])

    EARLY = 0

    # ---- Phase B: attention per head, software-pipelined over key slices ----
    DEPTH = 5  # software-pipeline offset between scores and ctx
    for h in range(NH):
        ft, ro = h // 2, (h % 2) * DH
        ctx_ps = ctx_pool.tile([P, S], f32, tag="ctx")
        pts = all_pts[h]

        def ctx_step(ks):
            pt = pts[ks]
            for qc4 in range(4):
                nc.tensor.matmul(
                    ctx_ps[0:VW, qc4 * 512:(qc4 + 1) * 512],
                    v_sb[:, ks * NH * VW + h * VW: ks * NH * VW + (h + 1) * VW],
                    pt[:, qc4 * 512:(qc4 + 1) * 512],
                    start=(ks == 0),
                    stop=(ks == KS - 1),
                )

        for ks in range(KS + DEPTH):
            if ks < KS and not (h == 0 and ks < EARLY):
                scores(h, ks)
            if ks >= DEPTH:
                ctx_step(ks - DEPTH)

        # normalize: ctxT[f, q] = ctx^T[f, q] / ctx^T[DH, q], in two column
        # chunks so the output projection can start on the first half.
        for nh2 in range(2):
            cs = slice(nh2 * (S // 2), (nh2 + 1) * (S // 2))
            rs = rs_pool.tile([1, S // 2], f32, tag="rs")
            nc.vector.reciprocal(rs[:], ctx_ps[DH:DH + 1, cs])
            rsb = rs_pool.tile([DH, S // 2], f32, tag="rsb")
            nc.gpsimd.partition_broadcast(rsb[:], rs[:])
            nc.vector.tensor_mul(ctxT[ft][ro:ro + DH, cs], ctx_ps[0:DH, cs], rsb[:])

    if dbg is not None:
        for half in range(2):
            nc.sync.dma_start(dbg["ctxT"].ap()[half * P:(half + 1) * P, :], ctxT[half][:])
    ctx_cm.__exit__(None, None, None)

    # ---- Phase D: output projection outT[o, q] = sum_f womT[f, o] * ctxT[f, q] ----
    # opj tiles come from ps_pool: its slots free up right after the last
    # scores' exp, so the first outproj matmuls don't wait on the ctx-pool
    # banks (which are only released once head 3's norm completes).
    out_cm = tc.tile_pool(name="out", bufs=4)
    out_pool = out_cm.__enter__()
    for qc in range(2):
        for ot in range(8):
            ps = ps_pool.tile([P, 1024], f32, tag="ps")
            for fs in range(2):
                for qh in range(2):
                    nc.tensor.matmul(
                        ps[:, qh * 512:(qh + 1) * 512],
                        wom_sb[:, fs * D + ot * P: fs * D + (ot + 1) * P],
                        ctxT[fs][:, qc * 1024 + qh * 512: qc * 1024 + (qh + 1) * 512],
                        start=(fs == 0),
                        stop=(fs == 1),
                    )
            osb = out_pool.tile([P, 1024], bf16, tag="osb")
            if (ot * 2 + qc) % 2 == 0:
                nc.vector.tensor_copy(osb[:], ps[:])
            else:
                nc.scalar.copy(osb[:], ps[:])
            for qh in range(2):
                nc.sync.dma_start(
                    t["outT"].ap()[ot * P:(ot + 1) * P, qc * 1024 + qh * 512: qc * 1024 + (qh + 1) * 512],
                    osb[:, qh * 512:(qh + 1) * 512],
                )
    out_cm.__exit__(None, None, None)
    es.close()


@functools.lru_cache(maxsize=1)
def _build(debug=False):
    import concourse.bacc as bacc
    import concourse.mybir as mybir
    import concourse.tile as tile

    bf16 = mybir.dt.bfloat16
    f32 = mybir.dt.float32

    nc = bacc.Bacc("TRN2", target_bir_lowering=False, debug=False, num_devices=NCORES)
    t = {
        "xqT": nc.dram_tensor("xqT", (D, S), bf16, kind="ExternalInput"),
        "xkT": nc.dram_tensor("xkT", (D, S), bf16, kind="ExternalInput"),
        "xvT": nc.dram_tensor("xvT", (D, S), bf16, kind="ExternalInput"),
        "wqT": nc.dram_tensor("wqT", (D, F), bf16, kind="ExternalInput"),
        "wkT": nc.dram_tensor("wkT", (D, F), bf16, kind="ExternalInput"),
        "wvT": nc.dram_tensor("wvT", (D, F), bf16, kind="ExternalInput"),
        "womT": nc.dram_tensor("womT", (F, D), bf16, kind="ExternalInput"),
        "nmT": nc.dram_tensor("nmT", (S, S), bf16, kind="ExternalInput"),
        "bq": nc.dram_tensor("bq", (F, 1), f32, kind="ExternalInput"),
        "bk": nc.dram_tensor("bk", (F, 1), f32, kind="ExternalInput"),
        "outT": nc.dram_tensor("outT", (D, S), bf16, kind="ExternalOutput"),
    }
    dbg = None
    if debug:
        dbg = {
            "qT": nc.dram_tensor("dbg_qT", (F, S), bf16, kind="ExternalOutput"),
            "kT": nc.dram_tensor("dbg_kT", (F, S), bf16, kind="ExternalOutput"),
            "v": nc.dram_tensor("dbg_v", (P, KS * NH * VW), bf16, kind="ExternalOutput"),
            "pT0": nc.dram_tensor("dbg_pT0", (S, S), bf16, kind="ExternalOutput"),
            "ctxT": nc.dram_tensor("dbg_ctxT", (F, S), bf16, kind="ExternalOutput"),
        }
    with tile.TileContext(nc) as tc:
        _emit(nc, tc, t, dbg)
    nc.compile()
    return nc


def _prep_core_inputs(c, Q, K, V, mask, Wq, bq, Wk, bk, Wv, Wo, _cache={}):
    import ml_dtypes

    bf = ml_dtypes.bfloat16
    b, g = divmod(c, GROUPS)
    bkey = ("batch", b, id(Q))
    if bkey not in _cache:
        _cache.clear()
        for bb in range(B):
            nm = 1.0 - mask[bb, 0].astype(np.float32)
            _cache[("batch", bb, id(Q))] = {
                "xqT": Q[bb].T.astype(bf),
                "xkT": K[bb].T.astype(bf),
                "xvT": V[bb].T.astype(bf),
                "nmT": nm.T.astype(bf),
            }
    fsl = slice(g * F, (g + 1) * F)
    return {
        **_cache[bkey],
        "wqT": Wq[fsl, :].T.astype(bf),
        "wkT": Wk[fsl, :].T.astype(bf),
        "wvT": Wv[fsl, :].T.astype(bf),
        "womT": Wo[:, fsl].T.astype(bf),
        "bq": bq[fsl].reshape(F, 1).astype(np.float32),
        "bk": bk[fsl].reshape(F, 1).astype(np.float32),
    }


def kernel(Q, K, V, mask, Wq, bq, Wk, bk, Wv, bv, Wo, bo, _trace=False, _tmpdir=None):
    from concourse.bass_utils import run_bass_kernel_spmd

    Q, K, V = np.asarray(Q, np.float32), np.asarray(K, np.float32), np.asarray(V, np.float32)
    mask = np.asarray(mask)
    Wq, Wk, Wv, Wo = (np.asarray(w, np.float32) for w in (Wq, Wk, Wv, Wo))
    bq, bk, bv, bo = (np.asarray(x, np.float32) for x in (bq, bk, bv, bo))

    nc = _build()
    in_maps = [_prep_core_inputs(c, Q, K, V, mask, Wq, bq, Wk, bk, Wv, Wo) for c in range(NCORES)]
    kw = {}
    if _trace:
        kw = dict(trace=True, tmpdir=_tmpdir)
    res = run_bass_kernel_spmd(nc, in_maps, core_ids=list(range(NCORES)), **kw)

    const = (Wo @ bv + bo).astype(np.float32)  # softmax rows sum to 1 -> bv enters linearly
    out = np.empty((B, S, D), np.float32)
    for b in range(B):
        acc = res.results[b * GROUPS]["outT"].astype(np.float32)
        for g in range(1, GROUPS):
            acc = acc + res.results[b * GROUPS + g]["outT"]
        out[b] = acc.T + const
    if _trace:
        kernel._last_results = res
    return out

